# revision 10
# baseline (speedup 1.0000x reference)
"""Causal self-attention kernel for 8 Trainium2 NeuronCores.

Problem: B=4, T=2048, C=1024, H=16 heads (D=64).
Sharding: data-parallel over batch (4) x tensor-parallel over heads (2 groups
of 8 heads). Core c handles batch c//2, head-group c%2. Each core computes
qkv for its 8 heads, full causal attention on TxT scores, and its partial
projection output; the host sums the two head-group partials per batch.

Device-side layout notes (per core):
  - activations live feature-major: xT [C, T], qT/kT pair-packed
    [128=(2 heads x 64d), pair, T], v as [T, jt, head, 65] (65th col = ones,
    which makes the PV matmul also produce the softmax denominator l).
  - scores are computed transposed (S.T = [j, i]) so the PV matmul needs no
    transposes; the causal mask is added (as -1e30) to the PSUM scores before
    exp; softmax normalization is deferred: y = y_u * (1/l) with one batched
    reciprocal per head pair and a DMA partition-broadcast of 1/l.
  - all matmuls run in bf16 with fp32 PSUM accumulation.
"""

import sys

if "/opt/trn_rl_repo" not in sys.path:
    sys.path.insert(0, "/opt/trn_rl_repo")

from contextlib import ExitStack

import ml_dtypes
import numpy as np

import concourse.bass as bass
import concourse.mybir as mybir
import concourse.tile as tile
from concourse.bass_utils import run_bass_kernel_spmd

BF16 = mybir.dt.bfloat16
F32 = mybir.dt.float32
NP_BF16 = ml_dtypes.bfloat16

P = 128
B, T, C = 4, 2048, 1024
H = 16
D = 64
HL = 8            # heads per core
NPAIR = HL // 2   # head pairs per core
NL = HL * D       # 512: local qkv width
CT = C // P       # 8 contraction tiles over C
DT = NL // P      # 4 contraction tiles over local head dims
NTO = C // P      # 8 output tiles for proj
TCH = T // 512    # 4 t-chunks
NJT = T // P      # 16 j tiles


def _split_excess_waits(nc, limit=1):
    """This walrus build supports a single sem-wait per instruction; move
    excess waits emitted by Tile onto preceding same-engine NoOps."""
    n = 0
    for bb in nc.main_func.blocks:
        out = []
        changed = False
        for inst in bb.instructions:
            si = inst.sync_info
            if si is not None and len(si.on_wait) > limit:
                waits = list(si.on_wait)
                excess, keep = waits[:-limit], waits[-limit:]
                for i in range(0, len(excess), limit):
                    out.append(
                        mybir.InstNoOp(
                            name=f"waitsplit_{n}",
                            ins=[],
                            outs=[],
                            engine=inst.engine,
                            sync_info=mybir.SyncInfo(
                                on_wait=excess[i : i + limit], on_update=[]
                            ),
                        )
                    )
                    n += 1
                si.on_wait = keep
                changed = True
            out.append(inst)
        if changed:
            bb.instructions = out
    return n


def build_nc(split_waits=True):
    nc = bass.Bass()
    AF = mybir.ActivationFunctionType

    xT = nc.dram_tensor("xT", [P, TCH, CT, 512], BF16, kind="ExternalInput")
    wq = nc.dram_tensor("wq", [P, CT, NL], BF16, kind="ExternalInput")
    wk = nc.dram_tensor("wk", [P, CT, NL], BF16, kind="ExternalInput")
    wv = nc.dram_tensor("wv", [P, CT, NL], BF16, kind="ExternalInput")
    wp = nc.dram_tensor("wp", [P, DT, C], BF16, kind="ExternalInput")
    bq = nc.dram_tensor("bq", [P, NPAIR], F32, kind="ExternalInput")
    bk = nc.dram_tensor("bk", [P, NPAIR], F32, kind="ExternalInput")
    bv = nc.dram_tensor("bv", [P, NL], F32, kind="ExternalInput")
    bp = nc.dram_tensor("bp", [P, NTO], F32, kind="ExternalInput")
    msk = nc.dram_tensor("msk", [P, P], F32, kind="ExternalInput")
    outT = nc.dram_tensor("outT", [P, NTO, T], F32, kind="ExternalOutput")

    with tile.TileContext(nc) as tc, ExitStack() as ctx:
        persist = ctx.enter_context(tc.tile_pool(name="persist", bufs=1))
        # one shared PSUM pool: tag "s" tiles are [128, 2, 512] (2 banks),
        # bufs=3 -> 6 banks; plus yA/yB accumulators (2 banks) = 8 banks.
        spsum = ctx.enter_context(tc.tile_pool(name="spsum", bufs=2, space="PSUM"))
        ypsum = ctx.enter_context(tc.tile_pool(name="ypsum", bufs=2, space="PSUM"))
        work = ctx.enter_context(tc.tile_pool(name="work", bufs=3))

        # ---- persistent SBUF tensors ----
        qT = persist.tile([P, NPAIR, T], BF16)   # [2x64d, pair, t]
        kT = persist.tile([P, NPAIR, T], BF16)
        vA = persist.tile([P, NJT, HL, D + 1], BF16)  # [j, jt, head, d|ones]
        yU = persist.tile([P, DT, T], BF16)  # unnormalized y.T, pair-packed
        yT = persist.tile([P, DT, T], BF16)      # normalized y.T
        lrow = persist.tile([P, 512], F32)   # l spread: pair pr rows 32*pr+..
        linv = persist.tile([P, 1, 512], F32)
        xs = persist.tile([P, TCH, CT, 512], BF16)
        wqs = persist.tile([P, CT, NL], BF16)
        wks = persist.tile([P, CT, NL], BF16)
        wvs = persist.tile([P, CT, NL], BF16)
        wps = persist.tile([P, DT, C], BF16)
        bqs = persist.tile([P, NPAIR], F32)
        bks = persist.tile([P, NPAIR], F32)
        bvs = persist.tile([P, NL], F32)
        bps = persist.tile([P, NTO], F32)
        msks = persist.tile([P, 1, P], F32)

        nc.scalar.dma_start(wvs[:], wv[:])
        nc.scalar.dma_start(wqs[:], wq[:])
        nc.scalar.dma_start(wks[:], wk[:])
        nc.scalar.dma_start(bqs[:], bq[:])
        nc.scalar.dma_start(bks[:], bk[:])
        nc.scalar.dma_start(bvs[:], bv[:])
        nc.scalar.dma_start(msks[:, 0, :], msk[:])
        nc.scalar.dma_start(wps[:], wp[:])
        nc.scalar.dma_start(bps[:], bp[:])

        nc.vector.memset(vA[:, :, :, D : D + 1], 1.0)

        def xsl(ct, t0, n):  # slice of xs covering [t0, t0+n) at c-tile ct
            tc_i, o = divmod(t0, 512)
            return xs[:, tc_i, ct, o : o + n]

        def emit_v(tt):
            ps = spsum.tile([P, 2, 512], F32, tag="s")
            for ct in range(CT):
                nc.tensor.matmul(
                    ps[:, 0, :],
                    lhsT=xsl(ct, tt * P, P),
                    rhs=wvs[:, ct, :],
                    start=(ct == 0),
                    stop=(ct == CT - 1),
                )
            nc.vector.tensor_tensor(
                out=vA[:, tt, :, 0:D],
                in0=ps[:, 0, :].rearrange("p (h d) -> p h d", h=HL),
                in1=bvs.rearrange("p (h d) -> p h d", h=HL),
                op=mybir.AluOpType.add,
            )

        def emit_qk(nt, tc_i):
            ps = spsum.tile([P, 2, 512], F32, tag="s")
            t_sl = slice(tc_i * 512, (tc_i + 1) * 512)
            for ct in range(CT):
                nc.tensor.matmul(
                    ps[:, 0, :],
                    lhsT=wqs[:, ct, nt * P : (nt + 1) * P],
                    rhs=xs[:, tc_i, ct, :],
                    start=(ct == 0),
                    stop=(ct == CT - 1),
                )
            for ct in range(CT):
                nc.tensor.matmul(
                    ps[:, 1, :],
                    lhsT=wks[:, ct, nt * P : (nt + 1) * P],
                    rhs=xs[:, tc_i, ct, :],
                    start=(ct == 0),
                    stop=(ct == CT - 1),
                )
            nc.vector.tensor_scalar(
                out=qT[:, nt, t_sl], in0=ps[:, 0, :],
                scalar1=bqs[:, nt : nt + 1], scalar2=None,
                op0=mybir.AluOpType.add,
            )
            nc.vector.tensor_scalar(
                out=kT[:, nt, t_sl], in0=ps[:, 1, :],
                scalar1=bks[:, nt : nt + 1], scalar2=None,
                op0=mybir.AluOpType.add,
            )

        # chunked x load interleaved with v and pair-0 q/k
        for tc_i in range(TCH):
            nc.sync.dma_start(xs[:, tc_i, :, :], xT[:, tc_i, :, :])
            for tt in range(4 * tc_i, 4 * tc_i + 4):
                emit_v(tt)
            emit_qk(0, tc_i)

        # ---- attention (emits q/k for pair p+1 interleaved) ----
        for pr in range(NPAIR):
            hA, hB = 2 * pr, 2 * pr + 1
            for ic in range(TCH):
                njt = 4 * ic + 4  # causal: j tiles 0 .. 4*ic+3
                i0 = ic * 512
                yA = ypsum.tile([D + 1, 512], F32, tag="yA")
                yB = ypsum.tile([D + 1, 512], F32, tag="yB")
                sts = {}

                def emit_scores(jt):
                    st = spsum.tile([P, 2, 512], F32, tag="s")
                    sts[jt] = st
                    ow = max(0, jt * P - i0)
                    j_sl = slice(jt * P, (jt + 1) * P)
                    i_sl = slice(i0 + ow, i0 + 512)
                    nc.tensor.matmul(
                        st[:, 0, ow:512],
                        lhsT=kT[0:D, pr, j_sl],
                        rhs=qT[0:D, pr, i_sl],
                        start=True, stop=True,
                        tile_position=(0, 0),
                    )
                    nc.tensor.matmul(
                        st[:, 1, ow:512],
                        lhsT=kT[D:P, pr, j_sl],
                        rhs=qT[D:P, pr, i_sl],
                        start=True, stop=True,
                        tile_position=(64, 0),
                    )
                    if jt >= 4 * ic:  # diagonal tile: add -1e30 above diag
                        nc.vector.tensor_tensor(
                            out=st[:, :, ow : ow + P],
                            in0=st[:, :, ow : ow + P],
                            in1=msks[:].to_broadcast([P, 2, P]),
                            op=mybir.AluOpType.add,
                        )

                emit_scores(0)
                if njt > 1:
                    emit_scores(1)
                for jt in range(njt):
                    st = sts.pop(jt)
                    ow = max(0, jt * P - i0)
                    pt = work.tile([P, 2, 512], BF16, tag="p")
                    nc.scalar.activation(
                        pt[:, :, ow:512], st[:, :, ow:512], AF.Exp, scale=0.125
                    )
                    if jt + 2 < njt:
                        emit_scores(jt + 2)
                    nc.tensor.matmul(
                        yA[:, ow:512],
                        lhsT=vA[:, jt, hA, :],
                        rhs=pt[:, 0, ow:512],
                        start=(jt == 0),
                        stop=(jt == njt - 1),
                    )
                    nc.tensor.matmul(
                        yB[:, ow:512],
                        lhsT=vA[:, jt, hB, :],
                        rhs=pt[:, 1, ow:512],
                        start=(jt == 0),
                        stop=(jt == njt - 1),
                    )
                # stash unnormalized y; stage l rows fp32 and lane-spread them
                i_sl = slice(i0, i0 + 512)
                nc.vector.tensor_copy(yU[0:D, pr, i_sl], yA[0:D, :])
                nc.vector.tensor_copy(yU[D:P, pr, i_sl], yB[0:D, :])
                lst = work.tile([P, 512], F32, tag="lb")
                nc.vector.tensor_copy(lst[D : D + 1, :], yA[D : D + 1, :])
                nc.vector.tensor_copy(lst[0:1, :], yB[D : D + 1, :])
                rA, rB = 32 * pr + ic, 32 * pr + TCH + ic
                nc.gpsimd.dma_start(lrow[rA : rA + 1, :], lst[D : D + 1, :])
                nc.gpsimd.dma_start(lrow[rB : rB + 1, :], lst[0:1, :])
                # overlap next pair's q/k with this pair's attention
                if pr + 1 < NPAIR:
                    emit_qk(pr + 1, ic)

            # batched normalization for this pair
            r0 = 32 * pr
            nc.vector.reciprocal(
                linv[r0 : r0 + 2 * TCH, 0, :], lrow[r0 : r0 + 2 * TCH, :]
            )
            for ic in range(TCH):
                i_sl = slice(ic * 512, (ic + 1) * 512)
                lb = work.tile([P, 512], F32, tag="lb")
                rA, rB = 32 * pr + ic, 32 * pr + TCH + ic
                nc.gpsimd.dma_start(
                    lb[0:D, :], linv[rA : rA + 1, :, :].to_broadcast([1, D, 512])
                )
                nc.gpsimd.dma_start(
                    lb[D:P, :], linv[rB : rB + 1, :, :].to_broadcast([1, D, 512])
                )
                nc.vector.tensor_tensor(
                    out=yT[:, pr, i_sl], in0=yU[:, pr, i_sl], in1=lb[:],
                    op=mybir.AluOpType.mult,
                )

        # ---- proj: out.T = wp.T @ yT + bp ----
        for nt in range(NTO):
            for tc_i in range(TCH):
                ps = spsum.tile([P, 2, 512], F32, tag="s")
                for dt in range(DT):
                    nc.tensor.matmul(
                        ps[:, 0, :],
                        lhsT=wps[:, dt, nt * P : (nt + 1) * P],
                        rhs=yT[:, dt, tc_i * 512 : (tc_i + 1) * 512],
                        start=(dt == 0),
                        stop=(dt == DT - 1),
                    )
                ot = work.tile([P, 512], F32, tag="out")
                nc.vector.tensor_scalar(
                    out=ot[:],
                    in0=ps[:, 0, :],
                    scalar1=bps[:, nt : nt + 1],
                    scalar2=None,
                    op0=mybir.AluOpType.add,
                )
                eng = nc.scalar if (nt + tc_i) % 2 == 0 else nc.sync
                eng.dma_start(outT[:, nt, tc_i * 512 : (tc_i + 1) * 512], ot[:])

    if split_waits:
        _split_excess_waits(nc, 1)
    return nc


def shard_inputs(x, w_attn, b_attn, w_proj, b_proj):
    """Build the 8 per-core input dicts (core = 2*batch + head_group)."""
    x = np.asarray(x, dtype=np.float32)
    w_attn = np.asarray(w_attn, dtype=np.float32)
    b_attn = np.asarray(b_attn, dtype=np.float32)
    w_proj = np.asarray(w_proj, dtype=np.float32)
    b_proj = np.asarray(b_proj, dtype=np.float32)

    # additive causal mask for a diagonal 128x128 block of S.T ([j, i]):
    # 0 where j <= i, -1e30 above the diagonal.
    pp = np.arange(P)
    msk = np.where(pp[:, None] <= pp[None, :], 0.0, -1e30).astype(np.float32)

    def wtile(w2d, ncols):  # [C_rows, ncols] -> [P, rows//P, ncols] bf16
        r = w2d.shape[0]
        return np.ascontiguousarray(
            w2d.reshape(r // P, P, ncols).transpose(1, 0, 2)
        ).astype(NP_BF16)

    in_maps = []
    for core in range(8):
        b, hg = divmod(core, 2)
        q0 = hg * NL
        xt = np.ascontiguousarray(x[b].T)  # [C, T]
        m = {
            "xT": np.ascontiguousarray(
                xt.reshape(CT, P, TCH, 512).transpose(1, 2, 0, 3)
            ).astype(NP_BF16),
            "wq": wtile(w_attn[:, q0 : q0 + NL], NL),
            "wk": wtile(w_attn[:, C + q0 : C + q0 + NL], NL),
            "wv": wtile(w_attn[:, 2 * C + q0 : 2 * C + q0 + NL], NL),
            "wp": wtile(w_proj[q0 : q0 + NL, :], C),
            "bq": np.ascontiguousarray(
                b_attn[q0 : q0 + NL].reshape(NPAIR, P).T
            ).astype(np.float32),
            "bk": np.ascontiguousarray(
                b_attn[C + q0 : C + q0 + NL].reshape(NPAIR, P).T
            ).astype(np.float32),
            "bv": np.broadcast_to(
                b_attn[2 * C + q0 : 2 * C + q0 + NL], (P, NL)
            ).astype(np.float32),
            "bp": (
                np.ascontiguousarray(b_proj.reshape(NTO, P).T).astype(np.float32)
                if hg == 0
                else np.zeros((P, NTO), np.float32)
            ),
            "msk": msk,
        }
        in_maps.append(m)
    return in_maps


def unshard_output(results):
    """Combine 8 per-core outT [P, NTO, T] partials into [B, T, C] fp32."""
    out = np.empty((B, T, C), dtype=np.float32)
    for b in range(B):
        acc = results[2 * b]["outT"] + results[2 * b + 1]["outT"]
        # [P, NTO, T] -> [C, T] -> [T, C]
        out[b] = acc.transpose(1, 0, 2).reshape(C, T).T
    return out


_NC_CACHE = {}


def kernel(x, w_attn, b_attn, w_proj, b_proj):
    if "nc" not in _NC_CACHE:
        _NC_CACHE["nc"] = build_nc()
    nc = _NC_CACHE["nc"]
    in_maps = shard_inputs(x, w_attn, b_attn, w_proj, b_proj)
    res = run_bass_kernel_spmd(nc, in_maps, core_ids=list(range(8)))
    return unshard_output(res.results)
